# revision 22
# baseline (speedup 1.0000x reference)
"""Trainium2 Bass kernel for nn_DetectionLoss (2-class detection loss).

Computes, over B=2^24 rows of logits [B,2] and labels [B]:
  ce    = mean(-log_softmax(outputs)[label])
  pred  = argmax(outputs, axis=1)
  confusion counts TP/TN/FP/FN from (label, pred)
  CS    = M[pred, label] with M = [[0,1],[0,0]]  -> mean(CS) = FN/B
  loss  = ce + coeff(TP,TN,FP,FN) * mean(CS)

Device math (2 classes): with d = x1 - x0 and h = label - 0.5:
  u       = d*h                  # sign-folded logit margin
  ce_row  = log1p(exp(-2u))      # ACT Exp then Ln (one table set), the
                                 # Ln pass accumulates the CE partial
  pred    = (d > 0), e = (u > 0) # e: prediction == label
Counts follow from three linear sums (n1 = sum(h) + B/2, p1 = sum(pred),
TP + TN = sum(e)):
  TP = (sum(e) + p1 + n1 - B) / 2, TN = sum(e) - TP,
  FP = p1 - TP, FN = n1 - TP.

Engine split per chunk (streamed through SBUF; the kernel is
HBM-bandwidth-bound at ~425 GB/s/core for 24 MiB per core):
  DMA : both inputs on the Sync HWDGE ring. Sync is a pure DMA-issue
        engine -- putting input DMAs on scalar/gpsimd head-of-line
        blocks ACT work / drags the SDMA round-robin (measured).
  DVE : d = x1 - x0 (1x, strided f32), h = lab - 0.5 (2x), u = d*h (2x),
        pred = d > 0 (4x), e = u > 0 (4x)
  ACT : t = Exp(-2u); ce = Ln(1+t) with fused accum -> per-chunk column
  PE  : ones^T @ {h, pred, e} slabs, 1024-wide (bf16 moving max),
        accumulated in PSUM. Big chunks accumulate into one PSUM group
        that is stopped and DVE-reduced BEFORE the trailing small
        chunks, so the reductions stay off the end-of-stream tail; the
        last small chunks use a separate tiny PSUM group.
All partials (CE columns + six count partials) leave in ONE small
output DMA; count arithmetic is exact in fp32 and combined on host.

Sharding: data-parallel over the batch dim across 8 NeuronCores.
"""

import numpy as np

import concourse.bass as bass
import concourse.mybir as mybir
import concourse.tile as tile
from concourse.bass_utils import run_bass_kernel_spmd

N_CORES = 8
P = 128
LAMBD = 0.5
MMN = 512  # matmul rhs free-dim tile (PSUM-bank limit for the f32 output)

_cache = {}

_MAX_WAITS = 1  # this walrus build rejects >1 embedded sync-wait per instruction


def _split_multiwaits(nc):
    """Walrus in this container can't encode instructions with multiple
    sync waits; hoist all but the last into standalone EventSemaphore
    waits on the same engine immediately before the instruction."""
    n = [0]

    def fix_block(blk):
        new_insts = []
        for ins in blk.instructions:
            si = ins.sync_info
            if si is not None and si.on_wait and len(si.on_wait) > _MAX_WAITS:
                waits = list(si.on_wait)
                for w in waits[: -_MAX_WAITS]:
                    n[0] += 1
                    ev = mybir.InstEventSemaphore(
                        name=f"I-waitsplit-{n[0]}",
                        ins=[],
                        outs=[],
                        sync_info=mybir.SyncInfo(on_wait=[w], on_update=[]),
                    )
                    ev.engine = ins.engine
                    new_insts.append(ev)
                si.on_wait = waits[-_MAX_WAITS:]
            new_insts.append(ins)
        blk.instructions = new_insts

    for fn in nc.m.functions:
        for blk in fn.blocks:
            fix_block(blk)


def _chunk_plan(rpp: int):
    """(plan, n_small): rows-per-partition per chunk, and how many
    trailing small chunks go to the late PSUM group. Flat 2048 chunks
    keep every DMA at line rate; the small trailing chunks shorten the
    post-stream tail."""
    if rpp % 2048 == 0 and rpp >= 8192:
        plan = [2048] * (rpp // 2048 - 1) + [1536, 512]
        return plan, 1
    # small test sizes: four equal chunks
    assert rpp % 4 == 0
    return [rpp // 4] * 4, 2


def _build(rows_per_core: int, lab64: bool):
    """Build the per-core Bass module. All cores run the same program on
    their own shard (pure data parallel, no collectives)."""
    key = (rows_per_core, lab64)
    if key in _cache:
        return _cache[key]

    assert rows_per_core % P == 0
    rpp = rows_per_core // P  # rows per partition
    plan, n_small = _chunk_plan(rpp)
    nch = len(plan)
    fmax = max(plan)
    smax = max(plan[nch - n_small :])
    # ce columns | big-group {h,p,e} sums | per-small-chunk {h,p,e} sums
    W = nch + 3 + 3 * n_small

    nc = bass.Bass(trn_type="TRN2")
    dtf = mybir.dt.float32
    dti = mybir.dt.int32
    dtb = mybir.dt.bfloat16
    Op = mybir.AluOpType
    Act = mybir.ActivationFunctionType

    LW = 2 if lab64 else 1  # int32 words per label
    x = nc.dram_tensor("x", [P, 2 * rpp], dtf, kind="ExternalInput")
    lab = nc.dram_tensor("lab", [P, LW * rpp], dti, kind="ExternalInput")
    acc = nc.dram_tensor("acc", [P, W], dtf, kind="ExternalOutput")

    with tile.TileContext(nc) as tc:
        with (
            tc.tile_pool(name="iox", bufs=5) as iox,
            tc.tile_pool(name="iol", bufs=5) as iol,
            tc.tile_pool(name="pef", bufs=4) as pef,
            tc.tile_pool(name="duo", bufs=2) as duo,
            tc.tile_pool(name="junk", bufs=1) as junk,
            tc.tile_pool(name="singles", bufs=1) as singles,
            tc.tile_pool(name="ps", bufs=1, space="PSUM") as psp,
        ):
            ones = singles.tile([P, 1], dtb)
            nc.vector.memset(ones, 1.0)
            ones_f = singles.tile([P, smax], dtb)
            nc.vector.memset(ones_f, 1.0)
            outbuf = singles.tile([P, W], dtf)
            jr = singles.tile([1, MMN], dtf)
            ps_big = [
                psp.tile([1, MMN], dtf, tag=f"psb{j}", name=f"ps_big{j}")
                for j in range(3)
            ]

            def mm_sums(tensors, c, F, c_lo, c_hi):
                """ones^T @ {h, pred, e} slabs into the big PSUM group."""
                nslab = (F + MMN - 1) // MMN
                for k in range(nslab):
                    sl = slice(k * MMN, min((k + 1) * MMN, F))
                    w = sl.stop - sl.start
                    first = c == c_lo and k == 0
                    last = c == c_hi and k == nslab - 1
                    for ps, tt in zip(ps_big, tensors):
                        nc.tensor.matmul(
                            ps[:, :w], ones, tt[:, sl], start=first, stop=last
                        )

            def drain(col0):
                """PSUM column sums -> three outbuf scalars, on ACT (Copy
                with fused accumulation) so DVE's in-order stream isn't
                blocked while the trailing small chunks arrive."""
                for j, ps in enumerate(ps_big):
                    nc.scalar.activation(
                        out=jr[:, :],
                        in_=ps,
                        func=Act.Copy,
                        accum_out=outbuf[0:1, col0 + j : col0 + j + 1],
                    )

            n_big = nch - n_small
            r0 = 0
            for c, F in enumerate(plan):
                r1 = r0 + F
                small = c >= n_big
                # labels first on the ring (small transfer, feeds h early)
                if lab64:
                    # int64 labels as little-endian int32 pairs; low word
                    # (stride 2) holds the value.
                    lt_full = iol.tile([P, LW * fmax], dti, tag="lt")
                    lt = lt_full[:, : LW * F]
                    nc.sync.dma_start(out=lt, in_=lab[:, LW * r0 : LW * r1])
                    lv = lt.rearrange("p (f two) -> p f two", two=2)[:, :, 0]
                else:
                    lt_full = iol.tile([P, fmax], dti, tag="lt")
                    lv = lt_full[:, :F]
                    nc.sync.dma_start(out=lv, in_=lab[:, r0:r1])
                xt_full = iox.tile([P, 2 * fmax], dtf, tag="xt")
                xt = xt_full[:, : 2 * F]
                nc.sync.dma_start(out=xt, in_=x[:, 2 * r0 : 2 * r1])
                xp = xt.rearrange("p (f two) -> p f two", two=2)

                h_full = pef.tile([P, fmax], dtb, tag="h")
                h = h_full[:, :F]
                # d and u share one tile, contiguously packed [d | u], so a
                # single 4x is_gt produces [pred | e] in one pass.
                du = duo.tile([P, 2 * fmax], dtb, tag="du")
                d = du[:, :F]
                u = du[:, F : 2 * F]
                pe2 = pef.tile([P, 2 * fmax], dtb, tag="pe2")
                pred = pe2[:, :F]
                e = pe2[:, F : 2 * F]
                of = ones_f[:, :F] if small else None
                cs = nch + 3 + 3 * (c - n_big)  # small-chunk sum columns
                # h = label - 0.5 in {-0.5,+0.5}; small chunks fuse the
                # per-partition sums into the op (no PE pass for them).
                if small:
                    nc.vector.scalar_tensor_tensor(
                        out=h, in0=lv, scalar=0.5, in1=of,
                        op0=Op.subtract, op1=Op.mult,
                        accum_out=outbuf[:, cs : cs + 1],
                    )
                else:
                    nc.vector.tensor_scalar(
                        out=h, in0=lv, scalar1=0.5, scalar2=None, op0=Op.subtract
                    )
                # d = x1 - x0
                nc.vector.tensor_sub(out=d, in0=xp[:, :, 1], in1=xp[:, :, 0])
                # u = d*h  (sign-folded logit margin; ce_row = log1p(exp(-2u)))
                nc.vector.tensor_mul(out=u, in0=d, in1=h)
                # pred = (d > 0), e = (u > 0)
                if small:
                    nc.vector.scalar_tensor_tensor(
                        out=pred, in0=d, scalar=0.0, in1=of,
                        op0=Op.is_gt, op1=Op.mult,
                        accum_out=outbuf[:, cs + 1 : cs + 2],
                    )
                    nc.vector.scalar_tensor_tensor(
                        out=e, in0=u, scalar=0.0, in1=of,
                        op0=Op.is_gt, op1=Op.mult,
                        accum_out=outbuf[:, cs + 2 : cs + 3],
                    )
                else:
                    nc.vector.tensor_scalar(
                        out=pe2[:, : 2 * F], in0=du[:, : 2 * F],
                        scalar1=0.0, scalar2=None, op0=Op.is_gt,
                    )

                # CE partial on ACT: t = exp(-2u); ce = ln(1+t), accum sum
                # into this chunk's column of outbuf.
                t_full = duo.tile([P, fmax], dtb, tag="t")
                t = t_full[:, :F]
                nc.scalar.activation(out=t, in_=u, func=Act.Exp, scale=-2.0)
                j3_full = junk.tile([P, fmax], dtb, tag="j3")
                j3 = j3_full[:, :F]
                nc.scalar.activation(
                    out=j3,
                    in_=t,
                    func=Act.Ln,
                    bias=1.0,
                    scale=1.0,
                    accum_out=outbuf[:, c : c + 1],
                )

                if not small:
                    mm_sums((h, pred, e), c, F, 0, n_big - 1)
                    if c == n_big - 1:
                        # big-group sums fold while the small chunks stream
                        drain(nch)
                r0 = r1

            nc.scalar.dma_start(out=acc[:], in_=outbuf)

    _cache[key] = (nc, nch)
    return nc, nch


def _combine(acc: np.ndarray, nch: int, B: int) -> np.ndarray:
    """Host-side scalar epilogue.

    acc: [n_cores, P, nch+3+3*n_small] f32 partials per core:
      cols 0..nch-1      : CE partial sums (per partition, per chunk)
      cols nch..nch+2    : big-group {sum(h), sum(pred), sum(e)} (row 0)
      then per small chunk: {sum(h), sum(pred), sum(e)} (per partition)
    Counts are exact (half-)integers in fp32 at every stage."""
    a = acc.astype(np.float64)
    CE = a[:, :, 0:nch].sum()
    n_small = (a.shape[2] - nch - 3) // 3
    hs = a[:, 0, nch].sum()
    p1 = a[:, 0, nch + 1].sum()
    C = a[:, 0, nch + 2].sum()
    for i in range(n_small):
        cs = nch + 3 + 3 * i
        hs += a[:, :, cs].sum()
        p1 += a[:, :, cs + 1].sum()
        C += a[:, :, cs + 2].sum()

    n1 = hs + B / 2.0  # labels == 1
    TP = (C + p1 + n1 - B) / 2.0
    TN = C - TP
    FP = p1 - TP
    FN = n1 - TP

    ce = CE / B
    mean_cs = FN / B
    nonzero = (TP > 0) and (TN > 0) and (FP > 0) and (FN > 0)
    ratio = (TP / max(TP + FN, 1.0)) * (FP / max(FP + TN, 1.0))
    if nonzero:
        coeff = -LAMBD * np.log(np.sqrt(max(ratio, 1e-30)))
    else:
        coeff = LAMBD
    return np.array(ce + coeff * mean_cs, dtype=np.float32)


def run(outputs: np.ndarray, labels: np.ndarray):
    """Run on 8 cores; returns (loss, BassKernelResults)."""
    outputs = np.asarray(outputs)
    labels = np.asarray(labels)
    B = outputs.shape[0]
    assert outputs.shape == (B, 2) and labels.shape == (B,)
    assert B % (N_CORES * P) == 0
    S = B // N_CORES
    rpp = S // P

    lab64 = labels.dtype.itemsize == 8
    nc, nch = _build(S, lab64)
    _split_multiwaits(nc)  # idempotent; CoreSim needs the unsplit module
    LW = 2 if lab64 else 1

    in_maps = []
    for i in range(N_CORES):
        xs = np.ascontiguousarray(outputs[i * S : (i + 1) * S], dtype=np.float32)
        xs = xs.reshape(P, 2 * rpp)
        ls = np.ascontiguousarray(labels[i * S : (i + 1) * S])
        ls = ls.view(np.int32).reshape(P, LW * rpp)
        in_maps.append({"x": xs, "lab": ls})

    res = run_bass_kernel_spmd(nc, in_maps, core_ids=list(range(N_CORES)))
    acc = np.stack([r["acc"] for r in res.results])
    return _combine(acc, nch, B), res


def kernel(outputs: np.ndarray, labels: np.ndarray) -> np.ndarray:
    return run(outputs, labels)[0]


# revision 26
# speedup vs baseline: 1.0261x; 1.0261x over previous
"""Trainium2 Bass kernel for nn_DetectionLoss (2-class detection loss).

Computes, over B=2^24 rows of logits [B,2] and labels [B]:
  ce    = mean(-log_softmax(outputs)[label])
  pred  = argmax(outputs, axis=1)
  confusion counts TP/TN/FP/FN from (label, pred)
  CS    = M[pred, label] with M = [[0,1],[0,0]]  -> mean(CS) = FN/B
  loss  = ce + coeff(TP,TN,FP,FN) * mean(CS)

Device math (2 classes): with d = x1 - x0 and h = label - 0.5:
  u       = d*h                  # sign-folded logit margin
  ce_row  = log1p(exp(-2u))      # ACT Exp then Ln (one table set), the
                                 # Ln pass accumulates the CE partial
  pred    = (d > 0), e = (u > 0) # e: prediction == label
Counts follow from three linear sums (n1 = sum(h) + B/2, p1 = sum(pred),
TP + TN = sum(e)):
  TP = (sum(e) + p1 + n1 - B) / 2, TN = sum(e) - TP,
  FP = p1 - TP, FN = n1 - TP.

Engine split per chunk (streamed through SBUF; the kernel is
HBM-bandwidth-bound at ~425 GB/s/core for 24 MiB per core):
  DMA : both inputs on the Sync HWDGE ring. Sync is a pure DMA-issue
        engine -- putting input DMAs on scalar/gpsimd head-of-line
        blocks ACT work / drags the SDMA round-robin (measured).
  DVE : d = x1 - x0 (1x, strided f32), h = lab - 0.5 (2x), u = d*h (2x),
        pred = d > 0 (4x), e = u > 0 (4x)
  ACT : t = Exp(-2u); ce = Ln(1+t) with fused accum -> per-chunk column
  PE  : ones^T @ {h, pred, e} slabs, 1024-wide (bf16 moving max),
        accumulated in PSUM. Big chunks accumulate into one PSUM group
        that is stopped and DVE-reduced BEFORE the trailing small
        chunks, so the reductions stay off the end-of-stream tail; the
        last small chunks use a separate tiny PSUM group.
All partials (CE columns + six count partials) leave in ONE small
output DMA; count arithmetic is exact in fp32 and combined on host.

Sharding: data-parallel over the batch dim across 8 NeuronCores.
"""

import numpy as np

import concourse.bass as bass
import concourse.mybir as mybir
import concourse.tile as tile
from concourse.bass_utils import run_bass_kernel_spmd

N_CORES = 8
P = 128
LAMBD = 0.5
MMN = 512  # matmul rhs free-dim tile (PSUM-bank limit for the f32 output)

_cache = {}

_MAX_WAITS = 1  # this walrus build rejects >1 embedded sync-wait per instruction


def _split_multiwaits(nc):
    """Walrus in this container can't encode instructions with multiple
    sync waits; hoist all but the last into standalone EventSemaphore
    waits on the same engine immediately before the instruction."""
    n = [0]

    def fix_block(blk):
        new_insts = []
        for ins in blk.instructions:
            si = ins.sync_info
            if si is not None and si.on_wait and len(si.on_wait) > _MAX_WAITS:
                waits = list(si.on_wait)
                for w in waits[: -_MAX_WAITS]:
                    n[0] += 1
                    ev = mybir.InstEventSemaphore(
                        name=f"I-waitsplit-{n[0]}",
                        ins=[],
                        outs=[],
                        sync_info=mybir.SyncInfo(on_wait=[w], on_update=[]),
                    )
                    ev.engine = ins.engine
                    new_insts.append(ev)
                si.on_wait = waits[-_MAX_WAITS:]
            new_insts.append(ins)
        blk.instructions = new_insts

    for fn in nc.m.functions:
        for blk in fn.blocks:
            fix_block(blk)


def _chunk_plan(rpp: int):
    """(plan, n_small): rows-per-partition per chunk, and how many
    trailing small chunks go to the late PSUM group. Flat 2048 chunks
    keep every DMA at line rate; the small trailing chunks shorten the
    post-stream tail."""
    if rpp % 2048 == 0 and rpp >= 8192:
        plan = [2048] * (rpp // 2048 - 2) + [1024, 1024, 1024, 512, 256, 256]
        n_small = 2
    else:
        # small test sizes: four equal chunks
        assert rpp % 4 == 0
        plan, n_small = [rpp // 4] * 4, 2
    assert sum(plan) == rpp and all(f % 4 == 0 for f in plan)
    return plan, n_small


def _build(rows_per_core: int, lab64: bool):
    """Build the per-core Bass module. All cores run the same program on
    their own shard (pure data parallel, no collectives)."""
    key = (rows_per_core, lab64)
    if key in _cache:
        return _cache[key]

    assert rows_per_core % P == 0
    rpp = rows_per_core // P  # rows per partition
    plan, n_small = _chunk_plan(rpp)
    nch = len(plan)
    fmax = max(plan)
    smax = max(plan[nch - n_small :])
    # ce columns | big-group {h,p,e} sums | per-small-chunk {h,p,e} sums
    W = nch + 3 + 3 * n_small

    nc = bass.Bass(trn_type="TRN2")
    dtf = mybir.dt.float32
    dti = mybir.dt.int32
    dtb = mybir.dt.bfloat16
    Op = mybir.AluOpType
    Act = mybir.ActivationFunctionType

    LW = 2 if lab64 else 1  # int32 words per label
    x = nc.dram_tensor("x", [P, 2 * rpp], dtf, kind="ExternalInput")
    lab = nc.dram_tensor("lab", [P, LW * rpp], dti, kind="ExternalInput")
    acc = nc.dram_tensor("acc", [P, W], dtf, kind="ExternalOutput")

    with tile.TileContext(nc) as tc:
        with (
            tc.tile_pool(name="iox", bufs=5) as iox,
            tc.tile_pool(name="iol", bufs=5) as iol,
            tc.tile_pool(name="pef", bufs=4) as pef,
            tc.tile_pool(name="duo", bufs=2) as duo,
            tc.tile_pool(name="junk", bufs=1) as junk,
            tc.tile_pool(name="singles", bufs=1) as singles,
            tc.tile_pool(name="ps", bufs=1, space="PSUM") as psp,
        ):
            ones = singles.tile([P, 1], dtb)
            nc.vector.memset(ones, 1.0)
            ones_f = singles.tile([P, smax], dtb)
            nc.vector.memset(ones_f, 1.0)
            outbuf = singles.tile([P, W], dtf)
            jr = singles.tile([1, MMN], dtf)
            ps_big = [
                psp.tile([1, MMN], dtf, tag=f"psb{j}", name=f"ps_big{j}")
                for j in range(3)
            ]

            def mm_sums(tensors, c, F, c_lo, c_hi):
                """ones^T @ {h, pred, e} slabs into the big PSUM group."""
                nslab = (F + MMN - 1) // MMN
                for k in range(nslab):
                    sl = slice(k * MMN, min((k + 1) * MMN, F))
                    w = sl.stop - sl.start
                    first = c == c_lo and k == 0
                    last = c == c_hi and k == nslab - 1
                    for ps, tt in zip(ps_big, tensors):
                        nc.tensor.matmul(
                            ps[:, :w], ones, tt[:, sl], start=first, stop=last
                        )

            def drain(col0):
                """PSUM column sums -> three outbuf scalars, on ACT (Copy
                with fused accumulation) so DVE's in-order stream isn't
                blocked while the trailing small chunks arrive."""
                for j, ps in enumerate(ps_big):
                    nc.scalar.activation(
                        out=jr[:, :],
                        in_=ps,
                        func=Act.Copy,
                        accum_out=outbuf[0:1, col0 + j : col0 + j + 1],
                    )

            n_big = nch - n_small
            r0 = 0
            for c, F in enumerate(plan):
                r1 = r0 + F
                small = c >= n_big
                # labels first on the ring (small transfer, feeds h early)
                if lab64:
                    # int64 labels as little-endian int32 pairs; low word
                    # (stride 2) holds the value.
                    lt_full = iol.tile([P, LW * fmax], dti, tag="lt")
                    lt = lt_full[:, : LW * F]
                    nc.sync.dma_start(out=lt, in_=lab[:, LW * r0 : LW * r1])
                    lv = lt.rearrange("p (f two) -> p f two", two=2)[:, :, 0]
                else:
                    lt_full = iol.tile([P, fmax], dti, tag="lt")
                    lv = lt_full[:, :F]
                    nc.sync.dma_start(out=lv, in_=lab[:, r0:r1])
                xt_full = iox.tile([P, 2 * fmax], dtf, tag="xt")
                xt = xt_full[:, : 2 * F]
                nc.sync.dma_start(out=xt, in_=x[:, 2 * r0 : 2 * r1])
                xp = xt.rearrange("p (f two) -> p f two", two=2)

                h_full = pef.tile([P, fmax], dtb, tag="h")
                h = h_full[:, :F]
                # d and u share one tile, contiguously packed [d | u], so a
                # single 4x is_gt produces [pred | e] in one pass.
                du = duo.tile([P, 2 * fmax], dtb, tag="du")
                d = du[:, :F]
                u = du[:, F : 2 * F]
                pe2 = pef.tile([P, 2 * fmax], dtb, tag="pe2")
                pred = pe2[:, :F]
                e = pe2[:, F : 2 * F]
                of = ones_f[:, :F] if small else None
                cs = nch + 3 + 3 * (c - n_big)  # small-chunk sum columns
                # h = label - 0.5 in {-0.5,+0.5}; small chunks fuse the
                # per-partition sums into the op (no PE pass for them).
                if small:
                    nc.vector.scalar_tensor_tensor(
                        out=h, in0=lv, scalar=0.5, in1=of,
                        op0=Op.subtract, op1=Op.mult,
                        accum_out=outbuf[:, cs : cs + 1],
                    )
                else:
                    nc.vector.tensor_scalar(
                        out=h, in0=lv, scalar1=0.5, scalar2=None, op0=Op.subtract
                    )
                # d = x1 - x0
                nc.vector.tensor_sub(out=d, in0=xp[:, :, 1], in1=xp[:, :, 0])
                # u = d*h  (sign-folded logit margin; ce_row = log1p(exp(-2u)))
                nc.vector.tensor_mul(out=u, in0=d, in1=h)
                # pred = (d > 0), e = (u > 0)
                if small:
                    nc.vector.scalar_tensor_tensor(
                        out=pred, in0=d, scalar=0.0, in1=of,
                        op0=Op.is_gt, op1=Op.mult,
                        accum_out=outbuf[:, cs + 1 : cs + 2],
                    )
                    nc.vector.scalar_tensor_tensor(
                        out=e, in0=u, scalar=0.0, in1=of,
                        op0=Op.is_gt, op1=Op.mult,
                        accum_out=outbuf[:, cs + 2 : cs + 3],
                    )
                else:
                    nc.vector.tensor_scalar(
                        out=pe2[:, : 2 * F], in0=du[:, : 2 * F],
                        scalar1=0.0, scalar2=None, op0=Op.is_gt,
                    )

                # CE partial on ACT: t = exp(-2u); ce = ln(1+t), accum sum
                # into this chunk's column of outbuf.
                t_full = duo.tile([P, fmax], dtb, tag="t")
                t = t_full[:, :F]
                nc.scalar.activation(out=t, in_=u, func=Act.Exp, scale=-2.0)
                j3_full = junk.tile([P, fmax], dtb, tag="j3")
                j3 = j3_full[:, :F]
                nc.scalar.activation(
                    out=j3,
                    in_=t,
                    func=Act.Ln,
                    bias=1.0,
                    scale=1.0,
                    accum_out=outbuf[:, c : c + 1],
                )

                if not small:
                    mm_sums((h, pred, e), c, F, 0, n_big - 1)
                    if c == n_big - 1:
                        # big-group sums fold while the small chunks stream
                        drain(nch)
                r0 = r1

            # NOTE: must issue from scalar (ACT): the ACTIVATION_READ_ACCUMULATOR
            # writes into outbuf are ordered before this DMA only by the scalar
            # engine's in-order stream; issuing from sync races them.
            nc.scalar.dma_start(out=acc[:], in_=outbuf)

    _cache[key] = (nc, nch)
    return nc, nch


def _combine(acc: np.ndarray, nch: int, B: int) -> np.ndarray:
    """Host-side scalar epilogue.

    acc: [n_cores, P, nch+3+3*n_small] f32 partials per core:
      cols 0..nch-1      : CE partial sums (per partition, per chunk)
      cols nch..nch+2    : big-group {sum(h), sum(pred), sum(e)} (row 0)
      then per small chunk: {sum(h), sum(pred), sum(e)} (per partition)
    Counts are exact (half-)integers in fp32 at every stage."""
    a = acc.astype(np.float64)
    CE = a[:, :, 0:nch].sum()
    n_small = (a.shape[2] - nch - 3) // 3
    hs = a[:, 0, nch].sum()
    p1 = a[:, 0, nch + 1].sum()
    C = a[:, 0, nch + 2].sum()
    for i in range(n_small):
        cs = nch + 3 + 3 * i
        hs += a[:, :, cs].sum()
        p1 += a[:, :, cs + 1].sum()
        C += a[:, :, cs + 2].sum()

    n1 = hs + B / 2.0  # labels == 1
    TP = (C + p1 + n1 - B) / 2.0
    TN = C - TP
    FP = p1 - TP
    FN = n1 - TP

    ce = CE / B
    mean_cs = FN / B
    nonzero = (TP > 0) and (TN > 0) and (FP > 0) and (FN > 0)
    ratio = (TP / max(TP + FN, 1.0)) * (FP / max(FP + TN, 1.0))
    if nonzero:
        coeff = -LAMBD * np.log(np.sqrt(max(ratio, 1e-30)))
    else:
        coeff = LAMBD
    return np.array(ce + coeff * mean_cs, dtype=np.float32)


def run(outputs: np.ndarray, labels: np.ndarray):
    """Run on 8 cores; returns (loss, BassKernelResults)."""
    outputs = np.asarray(outputs)
    labels = np.asarray(labels)
    B = outputs.shape[0]
    assert outputs.shape == (B, 2) and labels.shape == (B,)
    assert B % (N_CORES * P) == 0
    S = B // N_CORES
    rpp = S // P

    lab64 = labels.dtype.itemsize == 8
    nc, nch = _build(S, lab64)
    _split_multiwaits(nc)  # idempotent; CoreSim needs the unsplit module
    LW = 2 if lab64 else 1

    in_maps = []
    for i in range(N_CORES):
        xs = np.ascontiguousarray(outputs[i * S : (i + 1) * S], dtype=np.float32)
        xs = xs.reshape(P, 2 * rpp)
        ls = np.ascontiguousarray(labels[i * S : (i + 1) * S])
        ls = ls.view(np.int32).reshape(P, LW * rpp)
        in_maps.append({"x": xs, "lab": ls})

    res = run_bass_kernel_spmd(nc, in_maps, core_ids=list(range(N_CORES)))
    acc = np.stack([r["acc"] for r in res.results])
    return _combine(acc, nch, B), res


def kernel(outputs: np.ndarray, labels: np.ndarray) -> np.ndarray:
    return run(outputs, labels)[0]
